# revision 22
# baseline (speedup 1.0000x reference)
"""DigitalMapper kernel for 8 trn2 NeuronCores.

Math: reference computes  out = (x @ softmax(W, axis=1).T) > 0.5  with
x in {0,1}.  Let E = exp(W) (row-unnormalized).  Then

  out[b,o] > 0.5
    <=>  sum_i x[b,i]*E[o,i] / sum_i E[o,i] > 0.5
    <=>  sum_i (x[b,i] - 0.5) * E[o,i] > 0

so the softmax divide, the row-max subtraction and the per-column
threshold all fold into a single zero-threshold on a centered matmul.
(The row-max factor exp(m_o) scales a whole column positively - sign
is unchanged; |W| <= ~5.5 so exp never overflows fp32.)

Sharding: 4 batch-groups x 2 out-feature-groups across 8 cores.  Each
core gets x.T[:, bg*1024:...] and W.T[:, og*1024:...] (host does only
transpose/slice; subtract/exp/matmul/threshold all run on device) and
produces a [1024, 1024] block of the output.
"""

import sys

sys.path.insert(0, "/opt/trn_rl_repo")

import numpy as np

BATCH, IN_F, OUT_F = 4096, 2048, 2048
N_CORES = 8
BG, OG = 4, 2  # batch groups x out-feature groups
B_PER = BATCH // BG  # 1024 batch rows per core
O_PER = OUT_F // OG  # 1024 out features per core
P = 128
KT = IN_F // P  # 16 contraction tiles
MT = B_PER // P  # 8 output row tiles per core
NFREE = 512  # matmul moving free dim (one PSUM bank of fp32)
NO = O_PER // NFREE  # 2 n-chunks

_COMPILED = {}


def _patch_tile_drain():
    """walrus in this container allows only ONE sem-wait per CTRL (Drain/NOP)
    instruction; Tile's kernel-tail drain aggregates one wait per live
    semaphore.  Split the waits across a chain of SP nops."""
    import concourse.mybir as mybir
    import concourse.tile as tile_mod
    from concourse.vector_clock import ScopedClock

    if getattr(tile_mod.TileContext, "_drain_split_patched", False):
        return

    def _drain_and_barrier_split(self, tick_clock, wait_clock):
        nc = self.nc
        drain_inst = nc.sync.drain()
        wait_clock.add_sem_waits(
            drain_inst.ins, ScopedClock({None: tick_clock.global_clock})
        )
        si = drain_inst.ins.sync_info
        waits = list(si.on_wait) if si is not None else []
        if len(waits) > 1:
            si.on_wait.clear()
            si.on_wait.extend(waits[:1])
            for w in waits[1:]:
                nop = nc.sync.nop(nofuse=True)
                if nop.ins.sync_info is None:
                    nop.ins.sync_info = mybir.SyncInfo(on_wait=[], on_update=[])
                nop.ins.sync_info.on_wait.append(w)
        nc.all_engine_barrier()
        assert self.sems is not None
        popped = nc._tile_sem_poison_stack.pop()
        assert popped is self._sem_poison
        nc.clear_and_free_semaphores(list(self.sems.allocated().values()))
        nc.all_engine_barrier()

    tile_mod.TileContext._drain_and_barrier = _drain_and_barrier_split
    tile_mod.TileContext._drain_split_patched = True


def _split_multi_waits(nc):
    """walrus here allows very few sem-waits per instruction.  Hoist extra
    waits onto same-engine NOPs placed immediately before the instruction
    (same blocking point, engine executes in order).  DMA-queue instructions
    keep their waits - their sync runs through the DGE queues."""
    import concourse.mybir as mybir

    n = 0
    for f in nc.m.functions:
        for bb in f.blocks:
            new_insts = []
            for inst in bb.instructions:
                si = inst.sync_info
                if si is not None and si.on_wait and len(si.on_wait) > 1:
                    waits = list(si.on_wait)
                    si.on_wait.clear()
                    si.on_wait.append(waits[0])
                    for w in waits[1:]:
                        n += 1
                        new_insts.append(
                            mybir.InstNoOp(
                                name=f"wsplit-{n}",
                                opcode="NoOp",
                                engine=inst.engine,
                                sync_info=mybir.SyncInfo(on_wait=[w], on_update=[]),
                                bass_nofuse=True,
                            )
                        )
                new_insts.append(inst)
            if n:
                try:
                    bb.instructions[:] = new_insts
                except TypeError:
                    bb.instructions = new_insts
    return n


def _build(mm_dtype_name: str = "float32r", split_waits: bool = True,
           repeats: int = 1, correction=False):
    """One core's SPMD program.

    correction=False:  single fp32r matmul pass (PE ~56us/core).
    correction="bf16": fp32r pass on Ehi=round_f32r(exp(W)) plus a bf16
        pass on dE=exp(W)-Ehi (xb=+-1 is exact in both dtypes), which
        restores ~full-fp32 matmul accuracy at 2 cyc/row (PE ~110us).
    correction="f32r": same two-pass scheme but dE is kept in fp32r, so
        all four matmuls per (k,m) share ONE stationary operand (enables
        walrus LDW dedup) and the residual is even more precise.
    """
    if correction is True:
        correction = "bf16"
    import concourse.bass as bass
    import concourse.mybir as mybir
    import concourse.tile as tile

    _patch_tile_drain()

    f32 = mybir.dt.float32
    bf16 = mybir.dt.bfloat16
    u8 = mybir.dt.uint8
    mm_dt = getattr(mybir.dt, mm_dtype_name)
    Alu = mybir.AluOpType
    Act = mybir.ActivationFunctionType
    B2 = B_PER // 2  # batch columns per half

    nc = bass.Bass()
    xt = nc.dram_tensor("xt", [IN_F, B_PER], u8, kind="ExternalInput")
    wt = nc.dram_tensor("wt", [IN_F, O_PER], f32, kind="ExternalInput")
    out = nc.dram_tensor("out", [B_PER, O_PER], f32, kind="ExternalOutput")

    with tile.TileContext(nc) as tc:
        with (
            tc.tile_pool(name="xu", bufs=1) as xu_pool,
            tc.tile_pool(name="wr", bufs=3) as wr_pool,
            tc.tile_pool(name="eh", bufs=1) as eh_pool,
            tc.tile_pool(name="dl", bufs=1) as dl_pool,
            tc.tile_pool(name="xb", bufs=1) as xb_pool,
            tc.tile_pool(name="ps", bufs=1, space="PSUM") as ps_pool,
            tc.tile_pool(name="ot", bufs=3) as ot_pool,
        ):
          for _rep in range(repeats):
            if _rep == 0:
                # touch Exp immediately so the ~2.7us ACT table load
                # overlaps the first input DMAs instead of the first matmul
                warm = wr_pool.tile([P, 1], f32, name="warm", tag="warm")
                nc.vector.memset(warm[:], 0.0)
                nc.scalar.activation(warm[:], warm[:], Act.Exp)
            xu, ehi, dlo = [], [], []
            for k in range(KT):
                xk = xu_pool.tile([P, B_PER], u8, name=f"xu{k}", tag=f"xu{k}")
                nc.sync.dma_start(xk[:], xt[k * P : (k + 1) * P, :])
                xu.append(xk)
                wr = wr_pool.tile([P, O_PER], f32, name="wr", tag="wr")
                ek = eh_pool.tile([P, O_PER], mm_dt, name=f"e{k}", tag=f"e{k}")
                d_dt = bf16 if correction == "bf16" else mm_dt
                dk = (
                    dl_pool.tile([P, O_PER], d_dt, name=f"d{k}", tag=f"d{k}")
                    if correction
                    else None
                )
                # split the first weight tile into quarters so exp (and the
                # first matmuls) start ~1.2us after kernel launch instead of
                # waiting for the full 512KB row-block
                nq = 4 if k == 0 else 1
                qw = O_PER // nq
                for q in range(nq):
                    sl = slice(q * qw, (q + 1) * qw)
                    nc.sync.dma_start(wr[:, sl], wt[k * P : (k + 1) * P, sl])
                    if correction:
                        nc.scalar.activation(wr[:, sl], wr[:, sl], Act.Exp)
                        nc.scalar.copy(ek[:, sl], wr[:, sl])  # rounds -> f32r
                        nc.vector.tensor_tensor(
                            dk[:, sl], wr[:, sl], ek[:, sl], Alu.subtract
                        )
                    else:
                        nc.scalar.activation(ek[:, sl], wr[:, sl], Act.Exp)
                if correction:
                    dlo.append(dk)
                ehi.append(ek)

            for half in range(2):
                ms = range(half * 4, half * 4 + 4)
                xbr, xbb = [], []
                for k in range(KT):
                    xb_k = xb_pool.tile([P, B2], mm_dt, name=f"xb{k}", tag=f"xb{k}")
                    # x in {0,1} -> xb = 2x-1 in {-1,+1}, exact in any fp dtype
                    nc.vector.tensor_scalar(
                        xb_k[:], xu[k][:, half * B2 : (half + 1) * B2],
                        2.0, 1.0, Alu.mult, Alu.subtract,
                    )
                    xbr.append(xb_k)
                    if correction == "bf16":
                        xbb_k = xb_pool.tile(
                            [P, B2], bf16, name=f"xc{k}", tag=f"xc{k}"
                        )
                        nc.scalar.copy(xbb_k[:], xb_k[:])
                        xbb.append(xbb_k)

                pss = {}
                for m in ms:
                    pss[m] = ps_pool.tile(
                        [P, O_PER], f32, name=f"ps_{m % 4}", tag=f"ps_{m % 4}"
                    )

                def emit_mms(k, m):
                    lhsT = xbr[k][:, (m % 4) * P : (m % 4 + 1) * P]
                    for n in range(NO):
                        nc.tensor.matmul(
                            pss[m][:, n * NFREE : (n + 1) * NFREE],
                            lhsT,
                            ehi[k][:, n * NFREE : (n + 1) * NFREE],
                            start=(k == 0),
                            stop=(k == KT - 1 and not correction),
                        )
                    if correction:
                        src_xb = xbb if correction == "bf16" else xbr
                        lhsTb = src_xb[k][:, (m % 4) * P : (m % 4 + 1) * P]
                        for n in range(NO):
                            nc.tensor.matmul(
                                pss[m][:, n * NFREE : (n + 1) * NFREE],
                                lhsTb,
                                dlo[k][:, n * NFREE : (n + 1) * NFREE],
                                start=False,
                                stop=(k == KT - 1),
                            )

                def evict(m, pipelined=False):
                    otm = ot_pool.tile([P, O_PER], f32, name="otm", tag="otm")
                    row = half * 4 * P + (m % 4) * P
                    if pipelined:
                        # per-n-slice evict+store so the final DMA only
                        # trails the last psum bank, not the whole row
                        for n in range(NO):
                            sl = slice(n * NFREE, (n + 1) * NFREE)
                            nc.vector.tensor_scalar(
                                otm[:, sl], pss[m][:, sl], 0.0, None, Alu.is_gt
                            )
                            nc.sync.dma_start(out[row : row + P, sl], otm[:, sl])
                    else:
                        nc.vector.tensor_scalar(
                            otm[:], pss[m][:], 0.0, None, Alu.is_gt
                        )
                        nc.sync.dma_start(out[row : row + P, :], otm[:])

                if half == 0:
                    # k-outer: consume E[k] in DMA/exp arrival order
                    for k in range(KT):
                        for m in ms:
                            emit_mms(k, m)
                    for m in ms:
                        evict(m)
                else:
                    # all tiles resident now: m-outer so each m's psum
                    # finishes early and eviction/out-DMA pipelines
                    for m in ms:
                        for k in range(KT):
                            emit_mms(k, m)
                        evict(m, pipelined=True)

    if split_waits:
        _split_multi_waits(nc)
    return nc


def _get_compiled(mm_dtype_name: str = "float32r", correction=False):
    key = (mm_dtype_name, correction)
    if key not in _COMPILED:
        _COMPILED[key] = _build(mm_dtype_name, correction=correction)
    return _COMPILED[key]


def kernel(x: np.ndarray, raw_weight: np.ndarray, _mm_dtype: str = "float32r",
           _correction="f32r", _trace: bool = False):
    from concourse.bass_utils import run_bass_kernel_spmd

    nc = _get_compiled(_mm_dtype, _correction)

    # materialize as numpy first (inputs may arrive as jax arrays)
    x = np.asarray(x)
    raw_weight = np.asarray(raw_weight)

    # x is exactly 0.0/1.0; uint8 encodes it losslessly and quarters the DMA
    xT = np.ascontiguousarray(x.T.astype(np.uint8))
    wT = np.ascontiguousarray(raw_weight.T).astype(np.float32, copy=False)

    in_maps = []
    for c in range(N_CORES):
        bg, og = divmod(c, OG)
        in_maps.append(
            {
                "xt": np.ascontiguousarray(xT[:, bg * B_PER : (bg + 1) * B_PER]),
                "wt": np.ascontiguousarray(wT[:, og * O_PER : (og + 1) * O_PER]),
            }
        )

    res = run_bass_kernel_spmd(
        nc, in_maps, core_ids=list(range(N_CORES)), trace=_trace
    )

    full = np.empty((BATCH, OUT_F), dtype=x.dtype)
    for c in range(N_CORES):
        bg, og = divmod(c, OG)
        full[bg * B_PER : (bg + 1) * B_PER, og * O_PER : (og + 1) * O_PER] = (
            res.results[c]["out"]
        )
    if _trace:
        kernel.last_results = res
    return full
